# revision 1
# baseline (speedup 1.0000x reference)
"""BertLinearSelfAttention on 8 Trainium2 NeuronCores.

Problem (per reference):
  q = hs @ Wq.T + bq ; k = hs @ Wk.T + bk ; v = hs @ Wv.T + bv   (B,S,D)
  per head: scores = q @ k.T ; probs = scores * (mask >= 0) ; ctx = probs @ v
  B=2, S=2048, D=1024, H=16, HD=64. No softmax, binary key mask.

Sharding: core c = 4*b + g handles batch b and head group g (4 heads,
256 output features). Each core runs the same SPMD program on its own
slice; output is gathered host-side.

Algebraic moves:
  1) (scores * mask_k) @ v == scores @ (mask_k * v): the binary key mask
     applies to V rows instead of the S x S probs matrix.
  2) Masked keys contribute exactly zero, so K/V work only needs the
     valid keys. Inputs are compacted to CAP key slots (valid indices +
     zero-padding); a full-width fallback program handles the
     (astronomically unlikely) case of more than CAP valid keys.

On-chip layout (per core):
  xT blocks [128, 512]  hidden transposed via PE-identity transposes
  qT [256, S], kT [256, CAP] projection outputs kept feature-major
  v  [CAP, 256]  natural layout, bias via K=1 ones-matmul, pad mask
                 applied on the PSUM->SBUF copy
  scoresT pair tiles [s_k 128, 2 x s_q 512] = kT.T @ qT for both heads
                 of a pair (K=64 packed via disjoint PE row groups, two
                 PSUM banks), drained by one wide copy
  ctxT [128, s_q] both heads of a pair accumulated over s_k chunks via
                 col-packed fp16 matmuls (tile_position col groups)
Dtypes: x/weights/xT enter the PE as fp16 (eps 4.9e-4, on par with the
fp32r internal rounding); q/k are kept as fp32r so the scores matmuls
round only once; probs/v are fp16 for the col-packed ctx matmuls; all
PSUM accumulation is fp32. Measured end-to-end rel err ~6e-4.
"""
import numpy as np
import concourse.bass as bass
import concourse.mybir as mybir
import concourse.tile as tile
from concourse import bacc
from concourse.bass import ts
from concourse.bass_utils import run_bass_kernel_spmd

f32 = mybir.dt.float32
f32r = mybir.dt.float32r
bf16 = mybir.dt.bfloat16
fp16 = mybir.dt.float16
AF = mybir.ActivationFunctionType

B = 2
S = 2048
D = 1024
DL = 256          # output features per core (4 heads x 64)
KC = D // 128     # 8 contraction chunks
SC = S // 128     # 16 sequence chunks
MC = DL // 128    # 2 feature chunks / head pairs
SQW = 512         # attention s_q strip width
NSQ = S // SQW    # 4 strips
N_CORES = 8
CAP = 1152        # compacted key slots (valid count ~Binom(2048,.5), mean
                  # 1024 sd 22.6; 1152 is ~5.7 sigma up, fallback covers more)
CTX_BF16 = True   # probs/v in fp16 -> ctx pairs col-packed in the PE array
CTX_DT = mybir.dt.float16

_cache = {}


def _blocks(width):
    """Split `width` columns into 512-wide blocks (last may be shorter)."""
    out = []
    off = 0
    while off < width:
        w = min(512, width - off)
        out.append((off, w))
        off += w
    return out


def _build(compact):
    skv = (CAP if compact else S) // 128   # key chunks
    nc = bacc.Bacc("TRN2", target_bir_lowering=False, debug=False,
                   num_devices=N_CORES)
    X = nc.declare_dram_parameter("x", [S, D], fp16, isOutput=False)
    if compact:
        XKV = nc.declare_dram_parameter("xkv", [CAP, D], fp16, isOutput=False)
    IDN = nc.declare_dram_parameter("idn", [128, 128], fp16, isOutput=False)
    WQ = nc.declare_dram_parameter("wqt", [D, DL], fp16, isOutput=False)
    WK = nc.declare_dram_parameter("wkt", [D, DL], fp16, isOutput=False)
    WV = nc.declare_dram_parameter("wvt", [D, DL], fp16, isOutput=False)
    BQ = nc.declare_dram_parameter("bq2", [128, MC], f32, isOutput=False)
    BK = nc.declare_dram_parameter("bk2", [128, MC], f32, isOutput=False)
    BV = nc.declare_dram_parameter("bv", [1, DL], fp16, isOutput=False)
    ONE = nc.declare_dram_parameter("ones", [1, 128], fp16, isOutput=False)
    KVM = nc.declare_dram_parameter("kvm2", [128, skv], f32, isOutput=False)
    OUT = nc.declare_dram_parameter("out", [DL, S], f32, isOutput=True)

    with tile.TileContext(nc) as tc:
        with tc.tile_pool(name="sb", bufs=1) as sb, \
             tc.tile_pool(name="stg", bufs=4) as stg:

            ident = sb.tile([128, 128], fp16, tag="ident")
            nc.sync.dma_start(ident[:], IDN[:, :])

            qT = [sb.tile([128, S], f32r, tag=f"qT{m}", name=f"qT{m}")
                  for m in range(MC)]
            kT = [sb.tile([128, CAP if compact else S], f32r,
                          tag=f"kT{m}", name=f"kT{m}") for m in range(MC)]
            v_sb = sb.tile([128, skv * DL], CTX_DT if CTX_BF16 else f32r, tag="v_sb")

            eng = 0  # DVE/ACT alternator for PSUM->SBUF drains

            with tc.tile_pool(name="xs", bufs=8) as xs, \
                 tc.tile_pool(name="xtp", bufs=3) as xtp, \
                 tc.tile_pool(name="psA", bufs=5, space="PSUM") as psA:

                # first x block queued before the bulky weight loads
                xch0 = []
                for j in range(4):
                    xc = xs.tile([128, D], fp16, tag="xc")
                    nc.sync.dma_start(xc[:], X[ts(j, 128), :])
                    xch0.append(xc)

                wqt = sb.tile([128, KC * DL], fp16, tag="wqt")
                nc.sync.dma_start(wqt[:].rearrange("p (c m) -> p c m", c=KC),
                                  WQ.ap().rearrange("(c p) m -> p c m", p=128))
                wkt = sb.tile([128, KC * DL], fp16, tag="wkt")
                nc.sync.dma_start(wkt[:].rearrange("p (c m) -> p c m", c=KC),
                                  WK.ap().rearrange("(c p) m -> p c m", p=128))
                wvt = sb.tile([128, KC * DL], fp16, tag="wvt")
                nc.sync.dma_start(wvt[:].rearrange("p (c m) -> p c m", c=KC),
                                  WV.ap().rearrange("(c p) m -> p c m", p=128))
                bq2 = sb.tile([128, MC], f32, tag="bq2")
                nc.sync.dma_start(bq2[:], BQ[:, :])
                bk2 = sb.tile([128, MC], f32, tag="bk2")
                nc.sync.dma_start(bk2[:], BK[:, :])
                bv_t = sb.tile([1, DL], fp16, tag="bv")
                nc.sync.dma_start(bv_t[:], BV[:, :])
                ones_t = sb.tile([1, 128], fp16, tag="ones")
                nc.sync.dma_start(ones_t[:], ONE[:, :])
                kvm = sb.tile([128, skv], f32, tag="kvm")
                nc.sync.dma_start(kvm[:], KVM[:, :])

                def transpose_block(src_tiles, col0, width, kc, dst, dst_off):
                    """PE-transpose width cols of chunk tiles into dst."""
                    pt = psA.tile([128, 512], fp16, tag="tp", name="tp", bufs=3)
                    nw = width // 128
                    for j in range(nw):
                        nc.tensor.transpose(pt[:, ts(j, 128)],
                                            src_tiles[j][:, ts(kc, 128)],
                                            ident[:])
                    return pt

                def drain(dst_ap, src_ap, bias=None, scale=None, force=None):
                    nonlocal eng
                    e = eng if force is None else force
                    if e == 0:
                        if bias is not None:
                            nc.vector.tensor_scalar_add(dst_ap, src_ap, bias)
                        elif scale is not None:
                            nc.vector.tensor_scalar_mul(dst_ap, src_ap, scale)
                        else:
                            nc.vector.tensor_copy(dst_ap, src_ap)
                    else:
                        if bias is not None:
                            nc.scalar.add(dst_ap, src_ap, bias)
                        elif scale is not None:
                            nc.scalar.activation(dst_ap, src_ap, AF.Copy,
                                                 scale=scale)
                        else:
                            nc.scalar.copy(dst_ap, src_ap)
                    if force is None:
                        eng ^= 1

                # ---- A1: Q path over full x ------------------------------
                for bi, (off, w) in enumerate(_blocks(S)):
                    if bi == 0:
                        xch = xch0
                    else:
                        xch = []
                        for j in range(w // 128):
                            xc = xs.tile([128, D], fp16, tag="xc")
                            nc.sync.dma_start(xc[:],
                                              X[off + j * 128:off + (j + 1) * 128, :])
                            xch.append(xc)
                    xtb = []
                    for kc in range(KC):
                        pt = transpose_block(xch, off, w, kc, None, None)
                        xb = xtp.tile([128, 512], fp16, tag=f"xt{kc}",
                                      name=f"xt{kc}")
                        drain(xb[:, 0:w], pt[:, 0:w])
                        xtb.append(xb)
                    for mc in range(MC):
                        pt = psA.tile([128, 512], f32, tag="sc", name="qp")
                        for kc in range(KC):
                            nc.tensor.matmul(
                                pt[:, 0:w],
                                wqt[:, kc * DL + mc * 128:kc * DL + mc * 128 + 128],
                                xtb[kc][:, 0:w],
                                start=(kc == 0), stop=(kc == KC - 1))
                        drain(qT[mc][:, off:off + w], pt[:, 0:w],
                              bias=bq2[:, mc:mc + 1])
                    if not compact:
                        # K/V share the same transposed blocks
                        for mc in range(MC):
                            pt = psA.tile([128, 512], f32, tag="sc", name="kp")
                            for kc in range(KC):
                                nc.tensor.matmul(
                                    pt[:, 0:w],
                                    wkt[:, kc * DL + mc * 128:kc * DL + mc * 128 + 128],
                                    xtb[kc][:, 0:w],
                                    start=(kc == 0), stop=(kc == KC - 1))
                            drain(kT[mc][:, off:off + w], pt[:, 0:w],
                                  bias=bk2[:, mc:mc + 1])
                        for j in range(w // 128):
                            sc = (off + j * 128) // 128
                            pv = psA.tile([128, 512], f32, tag="sc", name="vp")
                            nc.tensor.matmul(pv[:, 0:DL], ones_t[:], bv_t[:],
                                             start=True, stop=False)
                            for kc in range(KC):
                                nc.tensor.matmul(pv[:, 0:DL],
                                                 xtb[kc][:, ts(j, 128)],
                                                 wvt[:, ts(kc, DL)],
                                                 start=False,
                                                 stop=(kc == KC - 1))
                            drain(v_sb[:, ts(sc, DL)], pv[:, 0:DL],
                                  scale=kvm[:, sc:sc + 1])

                # ---- A2 (compact): K/V over gathered keys ----------------
                if compact:
                    for off, w in _blocks(CAP):
                        xch = []
                        for j in range(w // 128):
                            xc = xs.tile([128, D], fp16, tag="xc")
                            nc.sync.dma_start(
                                xc[:],
                                XKV[off + j * 128:off + (j + 1) * 128, :])
                            xch.append(xc)
                        xtb = []
                        for kc in range(KC):
                            pt = transpose_block(xch, off, w, kc, None, None)
                            xb = xtp.tile([128, 512], fp16, tag=f"xt{kc}",
                                          name=f"xkvt{kc}")
                            drain(xb[:, 0:w], pt[:, 0:w])
                            xtb.append(xb)
                        for mc in range(MC):
                            pt = psA.tile([128, 512], f32, tag="sc", name="kp")
                            for kc in range(KC):
                                nc.tensor.matmul(
                                    pt[:, 0:w],
                                    wkt[:, kc * DL + mc * 128:kc * DL + mc * 128 + 128],
                                    xtb[kc][:, 0:w],
                                    start=(kc == 0), stop=(kc == KC - 1))
                            drain(kT[mc][:, off:off + w], pt[:, 0:w],
                                  bias=bk2[:, mc:mc + 1])
                        for j in range(w // 128):
                            sc = (off + j * 128) // 128
                            pv = psA.tile([128, 512], f32, tag="sc", name="vp")
                            nc.tensor.matmul(pv[:, 0:DL], ones_t[:], bv_t[:],
                                             start=True, stop=False)
                            for kc in range(KC):
                                nc.tensor.matmul(pv[:, 0:DL],
                                                 xtb[kc][:, ts(j, 128)],
                                                 wvt[:, ts(kc, DL)],
                                                 start=False,
                                                 stop=(kc == KC - 1))
                            drain(v_sb[:, ts(sc, DL)], pv[:, 0:DL],
                                  scale=kvm[:, sc:sc + 1])

            # ---- phase B: attention --------------------------------------
            pcnt = 0
            with tc.tile_pool(name="probs", bufs=skv + 6) as pp, \
                 tc.tile_pool(name="psB", bufs=3, space="PSUM") as psB, \
                 tc.tile_pool(name="psc", bufs=2, space="PSUM") as psc:
                for hp in range(MC):
                    for sq in range(NSQ):
                        pbs = []
                        for sk in range(skv):
                            spt = psB.tile([128, 1024], f32, tag="sc2")
                            nc.tensor.matmul(spt[:, 0:512],
                                             kT[hp][0:64, ts(sk, 128)],
                                             qT[hp][0:64, ts(sq, SQW)],
                                             start=True, stop=True)
                            nc.tensor.matmul(spt[:, 512:1024],
                                             kT[hp][64:128, ts(sk, 128)],
                                             qT[hp][64:128, ts(sq, SQW)],
                                             start=True, stop=True)
                            pb = pp.tile([128, 1024], CTX_DT if CTX_BF16 else f32r, tag="pb")
                            if eng == 0:
                                nc.vector.tensor_copy(pb[:], spt[:])
                            else:
                                nc.scalar.copy(pb[:], spt[:])
                            eng ^= 1
                            pbs.append(pb)
                        if CTX_BF16:
                            ct = psc.tile([128, SQW], f32, tag="ctx",
                                          name=f"ct{hp}_{sq}")
                            for sk in range(skv):
                                for h in range(2):
                                    nc.tensor.matmul(
                                        ct[h * 64:(h + 1) * 64, :],
                                        v_sb[:, sk * DL + hp * 128 + h * 64:
                                             sk * DL + hp * 128 + h * 64 + 64],
                                        pbs[sk][:, h * 512:(h + 1) * 512],
                                        start=(sk == 0), stop=(sk == skv - 1),
                                        tile_position=(0, h * 64),
                                        skip_group_check=True)
                            stage = stg.tile([128, SQW], f32, tag="st")
                            if eng == 0:
                                nc.vector.tensor_copy(stage[:], ct[:])
                            else:
                                nc.scalar.copy(stage[:], ct[:])
                            eng ^= 1
                        else:
                            cts = [psc.tile([64, SQW], f32, tag="ctx",
                                            name=f"ct{hp}_{sq}_{i}")
                                   for i in range(2)]
                            for sk in range(skv):
                                for h in range(2):
                                    nc.tensor.matmul(
                                        cts[h][:],
                                        v_sb[:, sk * DL + hp * 128 + h * 64:
                                             sk * DL + hp * 128 + h * 64 + 64],
                                        pbs[sk][:, h * 512:(h + 1) * 512],
                                        start=(sk == 0), stop=(sk == skv - 1))
                            stage = stg.tile([128, SQW], f32, tag="st")
                            nc.vector.tensor_copy(stage[0:64, :], cts[0][:])
                            nc.scalar.copy(stage[64:128, :], cts[1][:])
                        nc.sync.dma_start(
                            OUT[hp * 128:(hp + 1) * 128, ts(sq, SQW)], stage[:])

    nc.compile()
    return nc


def _get_nc(compact):
    key = "compact" if compact else "full"
    if key not in _cache:
        _cache[key] = _build(compact)
    return _cache[key]


def _make_in_maps(hidden_states, attention_mask, Wq, bq, Wk, bk, Wv, bv):
    hs = np.ascontiguousarray(np.asarray(hidden_states, dtype=np.float32))
    hs16 = hs.astype(np.float16)
    am = np.asarray(attention_mask, dtype=np.float32)

    # key compaction metadata per batch
    compact = True
    idxs, kvms, xkvs = [], [], []
    for b in range(B):
        valid = np.nonzero(am[b, 0, 0, :] >= 0)[0]
        if len(valid) > CAP:
            compact = False
            break
        idxp = np.zeros(CAP, np.int64)
        idxp[:len(valid)] = valid
        kvm = np.zeros(CAP, np.float32)
        kvm[:len(valid)] = 1.0
        idxs.append(idxp)
        kvms.append(kvm)
        xkvs.append(np.ascontiguousarray(hs16[b][idxp]))

    skv = (CAP if compact else S) // 128
    ones = np.ones((1, 128), np.float16)
    idn = np.eye(128, dtype=np.float16)
    in_maps = []
    for c in range(N_CORES):
        b, g = divmod(c, 4)
        sl = slice(g * DL, (g + 1) * DL)
        if compact:
            kvm2 = np.ascontiguousarray(kvms[b].reshape(skv, 128).T)
        else:
            kvm2 = np.ascontiguousarray(
                (am[b, 0, 0, :] >= 0).astype(np.float32).reshape(skv, 128).T)
        m = {
            "x": hs16[b],
            "idn": idn,
            "wqt": np.ascontiguousarray(np.asarray(Wq, np.float32)[sl, :].T.astype(np.float16)),
            "wkt": np.ascontiguousarray(np.asarray(Wk, np.float32)[sl, :].T.astype(np.float16)),
            "wvt": np.ascontiguousarray(np.asarray(Wv, np.float32)[sl, :].T.astype(np.float16)),
            "bq2": np.ascontiguousarray(
                np.asarray(bq, np.float32)[sl].reshape(MC, 128).T),
            "bk2": np.ascontiguousarray(
                np.asarray(bk, np.float32)[sl].reshape(MC, 128).T),
            "bv": np.ascontiguousarray(
                np.asarray(bv, np.float32)[sl].reshape(1, DL).astype(np.float16)),
            "ones": ones,
            "kvm2": kvm2,
        }
        if compact:
            m["xkv"] = xkvs[b]
        in_maps.append(m)
    return compact, in_maps


def _gather(results):
    out = np.empty((B, S, D), np.float32)
    for c in range(N_CORES):
        b, g = divmod(c, 4)
        out[b, :, g * DL:(g + 1) * DL] = results[c]["out"].T
    return out


def run_sharded(compact, in_maps, **kw):
    nc = _get_nc(compact)
    return run_bass_kernel_spmd(nc, in_maps, core_ids=list(range(N_CORES)), **kw)


def kernel(hidden_states, attention_mask, Wq, bq, Wk, bk, Wv, bv):
    compact, in_maps = _make_in_maps(hidden_states, attention_mask,
                                     Wq, bq, Wk, bk, Wv, bv)
    res = run_sharded(compact, in_maps)
    return _gather(res.results)



# revision 5
# speedup vs baseline: 2.0269x; 2.0269x over previous
"""BertLinearSelfAttention on 8 Trainium2 NeuronCores.

Problem (per reference):
  q = hs @ Wq.T + bq ; k = hs @ Wk.T + bk ; v = hs @ Wv.T + bv   (B,S,D)
  per head: scores = q @ k.T ; probs = scores * (mask >= 0) ; ctx = probs @ v
  B=2, S=2048, D=1024, H=16, HD=64. No softmax, binary key mask.

Key algebraic move: with no softmax, the attention is associative:
  ctx_h = (q_h k_h^T * mask) v_h = q_h @ M_h,   M_h = k_h^T diag(mask) v_h
M_h is only [64, 64] per head, so the O(S^2) scores/probs work disappears.
Remaining FLOPs are the q/k/v projections plus tiny M and M-apply matmuls.
Masked keys contribute nothing, so k/v are computed over host-compacted
valid keys only (CAP slots, zero-padded; full-width fallback otherwise).

Sharding: core c = 4*b + g handles batch b and head group g (4 heads,
256 output features). Pure SPMD, no collectives; host gathers the output.

Per-core device program (all matmul operands fp16 except M-path f32r):
  qT [256, S]    = Wq_g hs^T           (xt: host chunk-transposed x)
  kv [CAP, 512]  = xkv [Wk_g^T|Wv_g^T] (xkvt: host chunk-transposed xkv)
  M2_p [128,128] = kv_k(pair p)^T @ kv_v(pair p)   accumulated over CAP
                   (diag 64x64 blocks are M_h; off-diag cross terms dropped)
  ctxT [256, S]  = blockdiag(M)^T @ qT  -> OUT [256, S] fp16

Measured end-to-end rel err ~5e-4 (fp16 inputs, fp32 PSUM accumulation).
"""
import numpy as np
import concourse.bass as bass
import concourse.mybir as mybir
import concourse.tile as tile
from concourse import bacc
from concourse.bass_utils import run_bass_kernel_spmd

f32 = mybir.dt.float32
f32r = mybir.dt.float32r
fp16 = mybir.dt.float16
AF = mybir.ActivationFunctionType

B = 2
S = 2048
D = 1024
DL = 256          # output features per core (4 heads x 64)
KC = D // 128     # 8 contraction chunks
MC = DL // 128    # 2 feature chunks (head pairs)
NSB = S // 512    # 4 query strips
N_CORES = 8
CAP = 1152        # compacted key slots (valid ~Binom(2048,.5): mean 1024,
                  # sd 22.6; 1152 is ~5.7 sigma; fallback covers more)

_cache = {}


def _build(compact, with_bias):
    skv = (CAP if compact else S) // 128   # key chunks
    use_kvm = (not compact) or with_bias   # per-key scale on kv drains
    nc = bacc.Bacc("TRN2", target_bir_lowering=False, debug=False,
                   num_devices=N_CORES)
    XT = nc.declare_dram_parameter("xt", [128, KC * S], fp16, isOutput=False)
    XKVT = nc.declare_dram_parameter("xkvt", [128, skv * D], fp16,
                                     isOutput=False)
    WQT = nc.declare_dram_parameter("wqt", [128, KC * DL], fp16,
                                    isOutput=False)
    WKV = nc.declare_dram_parameter("wkv", [128, KC * 2 * DL], fp16,
                                    isOutput=False)
    if with_bias:
        BQ2 = nc.declare_dram_parameter("bq2", [128, MC], f32, isOutput=False)
        BKV = nc.declare_dram_parameter("bkv", [1, 2 * DL], fp16,
                                        isOutput=False)
        ONE = nc.declare_dram_parameter("ones1", [1, 128], fp16,
                                        isOutput=False)
    if use_kvm:
        KVM = nc.declare_dram_parameter("kvm2", [128, skv], f32,
                                        isOutput=False)
    OUT = nc.declare_dram_parameter("out", [DL, S], fp16, isOutput=True)

    with tile.TileContext(nc) as tc:
        with tc.tile_pool(name="sb", bufs=1) as sb, \
             tc.tile_pool(name="stg", bufs=4) as stg:

            wq_sb = sb.tile([128, KC * DL], fp16, tag="wq")
            xt_sb = sb.tile([128, KC * S], fp16, tag="xt")
            wkv_sb = sb.tile([128, KC * 2 * DL], fp16, tag="wkv")
            xkvt_sb = sb.tile([128, skv * D], fp16, tag="xkvt")
            qT = [sb.tile([128, S], f32r, tag=f"qT{p}", name=f"qT{p}")
                  for p in range(MC)]
            kv_sb = sb.tile([128, skv * 2 * DL], f32r, tag="kv")
            m2s = [sb.tile([128, 128], f32r, tag=f"m2s{p}", name=f"m2s{p}")
                   for p in range(MC)]
            if with_bias:
                bq2 = sb.tile([128, MC], f32, tag="bq2")
                bkv = sb.tile([1, 2 * DL], fp16, tag="bkv")
                ones1 = sb.tile([1, 128], fp16, tag="ones1")
            if use_kvm:
                kvm = sb.tile([128, skv], f32, tag="kvm")

            # DMA order matches PE consumption: q path first, then kv.
            nc.sync.dma_start(wq_sb[:], WQT[:, :])
            if with_bias:
                nc.sync.dma_start(bq2[:], BQ2[:, :])
                nc.sync.dma_start(bkv[:], BKV[:, :])
                nc.sync.dma_start(ones1[:], ONE[:, :])
            if use_kvm:
                nc.sync.dma_start(kvm[:], KVM[:, :])
            xtq = KC * S // 4
            for i in range(2):
                nc.sync.dma_start(xt_sb[:, i * xtq:(i + 1) * xtq],
                                  XT[:, i * xtq:(i + 1) * xtq])
            nc.sync.dma_start(wkv_sb[:], WKV[:, :])
            for i in range(2, 4):
                nc.sync.dma_start(xt_sb[:, i * xtq:(i + 1) * xtq],
                                  XT[:, i * xtq:(i + 1) * xtq])
            kq = skv * D // 3
            for i in range(3):
                nc.sync.dma_start(xkvt_sb[:, i * kq:(i + 1) * kq],
                                  XKVT[:, i * kq:(i + 1) * kq])

            eng = 0  # DVE/ACT alternator for PSUM->SBUF drains

            def drain(dst_ap, src_ap, bias=None, scale=None):
                nonlocal eng
                if eng == 0:
                    if bias is not None:
                        nc.vector.tensor_scalar_add(dst_ap, src_ap, bias)
                    elif scale is not None:
                        nc.vector.tensor_scalar_mul(dst_ap, src_ap, scale)
                    else:
                        nc.vector.tensor_copy(dst_ap, src_ap)
                else:
                    if bias is not None:
                        nc.scalar.add(dst_ap, src_ap, bias)
                    elif scale is not None:
                        nc.scalar.activation(dst_ap, src_ap, AF.Copy,
                                             scale=scale)
                    else:
                        nc.scalar.copy(dst_ap, src_ap)
                eng ^= 1

            # ---- Q projection: qT[mc] [128, S] = Wq_mc @ x^T -------------
            with tc.tile_pool(name="psq", bufs=1, space="PSUM") as psq:
                qps = [psq.tile([128, 512], f32, tag=f"q{i}", name=f"qps{i}")
                       for i in range(MC * NSB)]
                for kc in range(KC):
                    for mc in range(MC):
                        w = wq_sb[:, kc * DL + mc * 128:kc * DL + mc * 128 + 128]
                        for sbk in range(NSB):
                            nc.tensor.matmul(
                                qps[mc * NSB + sbk][:], w,
                                xt_sb[:, kc * S + sbk * 512:kc * S + sbk * 512 + 512],
                                start=(kc == 0), stop=(kc == KC - 1))
                for mc in range(MC):
                    for sbk in range(NSB):
                        drain(qT[mc][:, sbk * 512:(sbk + 1) * 512],
                              qps[mc * NSB + sbk][:],
                              bias=bq2[:, mc:mc + 1] if with_bias else None)

            # ---- K/V projection over (compacted) keys --------------------
            with tc.tile_pool(name="psk", bufs=3, space="PSUM") as psk:
                for j in range(skv):
                    pv = psk.tile([128, 512], f32, tag="kv")
                    if with_bias:
                        nc.tensor.matmul(pv[:], ones1[:], bkv[:],
                                         start=True, stop=False)
                    for kc in range(KC):
                        nc.tensor.matmul(
                            pv[:],
                            xkvt_sb[:, j * D + kc * 128:j * D + (kc + 1) * 128],
                            wkv_sb[:, kc * 512:(kc + 1) * 512],
                            start=(kc == 0 and not with_bias),
                            stop=(kc == KC - 1))
                    drain(kv_sb[:, j * 512:(j + 1) * 512], pv[:],
                          scale=kvm[:, j:j + 1] if use_kvm else None)

            # ---- M pair tiles: M2_p = k_p^T @ v_p over keys --------------
            with tc.tile_pool(name="psm", bufs=2, space="PSUM") as psm:
                for p in range(MC):
                    mp = psm.tile([128, 128], f32, tag="m2")
                    for j in range(skv):
                        nc.tensor.matmul(
                            mp[:],
                            kv_sb[:, j * 512 + p * 128:j * 512 + (p + 1) * 128],
                            kv_sb[:, j * 512 + 256 + p * 128:
                                  j * 512 + 256 + (p + 1) * 128],
                            start=(j == 0), stop=(j == skv - 1))
                    # keep only the per-head diagonal blocks; zero the
                    # cross-head off-diagonal quadrants
                    nc.vector.tensor_copy(m2s[p][0:64, 0:64], mp[0:64, 0:64])
                    nc.scalar.copy(m2s[p][64:128, 64:128],
                                   mp[64:128, 64:128])
                    nc.vector.tensor_scalar_mul(m2s[p][0:64, 64:128],
                                                mp[0:64, 64:128], 0.0)
                    nc.vector.tensor_scalar_mul(m2s[p][64:128, 0:64],
                                                mp[64:128, 0:64], 0.0)

            # ---- apply M: ctxT[p] [128, S] = blockdiag(M)^T @ qT[p] ------
            with tc.tile_pool(name="psc", bufs=4, space="PSUM") as psc:
                for p in range(MC):
                    for sbk in range(NSB):
                        cp = psc.tile([128, 512], f32, tag="ctx")
                        nc.tensor.matmul(cp[:], m2s[p][:],
                                         qT[p][:, sbk * 512:(sbk + 1) * 512],
                                         start=True, stop=True)
                        st = stg.tile([128, 512], fp16, tag="st")
                        drain(st[:], cp[:])
                        nc.sync.dma_start(
                            OUT[p * 128:(p + 1) * 128,
                                sbk * 512:(sbk + 1) * 512], st[:])

    nc.compile()
    return nc


def _get_nc(compact, with_bias):
    key = (compact, with_bias)
    if key not in _cache:
        _cache[key] = _build(compact, with_bias)
    return _cache[key]


def _chunkT(a):
    """[R, D] row-major -> [128, (D//128)*R]: out[p, kc*R + r] = a[r, kc*128+p]."""
    R, Din = a.shape
    return np.ascontiguousarray(
        a.T.reshape(Din // 128, 128, R).transpose(1, 0, 2).reshape(128, -1))


def _make_in_maps(hidden_states, attention_mask, Wq, bq, Wk, bk, Wv, bv):
    hs16 = np.asarray(hidden_states, dtype=np.float32).astype(np.float16)
    am = np.asarray(attention_mask, dtype=np.float32)
    bq = np.asarray(bq, np.float32)
    bk = np.asarray(bk, np.float32)
    bv = np.asarray(bv, np.float32)
    Wq = np.asarray(Wq, np.float32)
    Wk = np.asarray(Wk, np.float32)
    Wv = np.asarray(Wv, np.float32)
    with_bias = bool(bq.any() or bk.any() or bv.any())

    compact = True
    xkvts, kvms = [], []
    for b in range(B):
        valid = np.nonzero(am[b, 0, 0, :] >= 0)[0]
        if len(valid) > CAP:
            compact = False
            break
        xkv = np.zeros((CAP, D), np.float16)
        xkv[:len(valid)] = hs16[b][valid]
        xkvts.append(xkv)
        kvmv = np.zeros(CAP, np.float32)
        kvmv[:len(valid)] = 1.0
        kvms.append(kvmv)

    skv = (CAP if compact else S) // 128
    use_kvm = (not compact) or with_bias

    xts = [_chunkT(hs16[b]) for b in range(B)]   # [128, KC*S] per batch
    if compact:
        xkvt_blk = []
        for b in range(B):
            xkv = xkvts[b]
            blocks = [_chunkT(xkv[j * 128:(j + 1) * 128]) for j in range(skv)]
            xkvt_blk.append(np.ascontiguousarray(np.concatenate(blocks, 1)))
    else:
        xkvt_blk = []
        for b in range(B):
            blocks = [_chunkT(hs16[b][j * 128:(j + 1) * 128])
                      for j in range(skv)]
            xkvt_blk.append(np.ascontiguousarray(np.concatenate(blocks, 1)))

    in_maps = []
    for c in range(N_CORES):
        b, g = divmod(c, 4)
        sl = slice(g * DL, (g + 1) * DL)
        wq_sel = Wq[sl].astype(np.float16)            # [256, 1024]
        wkv_sel = np.vstack([Wk[sl], Wv[sl]]).astype(np.float16)  # [512, 1024]
        m = {
            "xt": xts[b],
            "xkvt": xkvt_blk[b],
            # [p, kc*DL + f] = W[f, kc*128+p]
            "wqt": np.ascontiguousarray(
                wq_sel.T.reshape(KC, 128, DL).transpose(1, 0, 2)
                .reshape(128, KC * DL)),
            "wkv": np.ascontiguousarray(
                wkv_sel.T.reshape(KC, 128, 2 * DL).transpose(1, 0, 2)
                .reshape(128, KC * 2 * DL)),
        }
        if with_bias:
            m["bq2"] = np.ascontiguousarray(
                bq[sl].reshape(MC, 128).T.astype(np.float32))
            m["bkv"] = np.ascontiguousarray(
                np.concatenate([bk[sl], bv[sl]]).reshape(1, 2 * DL)
                .astype(np.float16))
            m["ones1"] = np.ones((1, 128), np.float16)
        if use_kvm:
            if compact:
                kvmv = kvms[b]
            else:
                kvmv = (am[b, 0, 0, :] >= 0).astype(np.float32)
            m["kvm2"] = np.ascontiguousarray(kvmv.reshape(skv, 128).T)
        in_maps.append(m)
    return (compact, with_bias), in_maps


def _gather(results):
    out = np.empty((B, S, D), np.float32)
    for c in range(N_CORES):
        b, g = divmod(c, 4)
        out[b, :, g * DL:(g + 1) * DL] = results[c]["out"].T.astype(np.float32)
    return out


def run_sharded(variant, in_maps, **kw):
    compact, with_bias = variant if isinstance(variant, tuple) else (variant, False)
    nc = _get_nc(compact, with_bias)
    return run_bass_kernel_spmd(nc, in_maps, core_ids=list(range(N_CORES)), **kw)


def kernel(hidden_states, attention_mask, Wq, bq, Wk, bk, Wv, bv):
    variant, in_maps = _make_in_maps(hidden_states, attention_mask,
                                     Wq, bq, Wk, bk, Wv, bv)
    res = run_sharded(variant, in_maps)
    return _gather(res.results)
